# revision 17
# baseline (speedup 1.0000x reference)
"""Bass/Trainium2 kernel for nn_DenseMap (bilinear grid-sample embedding lookup).

Strategy: shard the 128 maps across 8 NeuronCores (16 maps each, in 2 phases
of 8). Table relayout (host): per map, 8 feature columns x 2 x-shifts, each
stored as even-start and odd-start y-pair streams so ONE ap_gather index with
d=2 fetches all 4 bilinear neighbors x 8 features across the 16 SBUF
partitions owned by one GPSIMD core. Device: ap_gather + DVE lerps +
stream_shuffle + PE select-transpose merge, DMA out sample-major.
"""
import sys, os
sys.path.insert(0, "/opt/trn_rl_repo")
import numpy as np

FEAT = 8
RES = 128
OFF = RES * RES          # 16384 grid pts / map
MAPS = 128
B = 32768
NCORES = 8
MP_NC = 16               # maps per NeuronCore
PH_M = 8                 # maps per phase
S = 2048                 # samples per chunk
NCH = B // S             # 32 chunks per phase
NPAIR = OFF              # num_elems for ap_gather (8192 E-pairs + 8192 O-pairs)

_cache = {}


def _build_program():
    import concourse.bass as bass
    import concourse.tile as tile
    from concourse import bacc, mybir

    nc = bacc.Bacc("TRN2", target_bir_lowering=False, debug=False,
                   num_devices=NCORES)
    dt = mybir.dt
    bf = dt.bfloat16
    emb_d = [nc.dram_tensor(f"emb{p}", [128, 2 * NPAIR], bf,
                            kind="ExternalInput").ap() for p in range(2)]
    idx_d = [nc.dram_tensor(f"idx{p}", [128, (S // 16) * NCH], dt.int16,
                            kind="ExternalInput").ap() for p in range(2)]
    wxa_d = [nc.dram_tensor(f"wxa{p}", [128, B], bf,
                            kind="ExternalInput").ap() for p in range(2)]
    wya_d = [nc.dram_tensor(f"wya{p}", [128, B], bf,
                            kind="ExternalInput").ap() for p in range(2)]
    p1_d = nc.dram_tensor("p1", [128, 80], bf, kind="ExternalInput").ap()
    p2_d = nc.dram_tensor("p2", [128, 80], bf, kind="ExternalInput").ap()
    p3_d = nc.dram_tensor("p3", [128, 80], bf, kind="ExternalInput").ap()
    out_d = nc.dram_tensor("out", [2, NCH, 128, (S // 128) * 80], dt.float32,
                           kind="ExternalOutput").ap()

    # stream_shuffle mask: within each 32-partition quadrant, rows 0..7 <- 8..15,
    # rows 16..23 <- 24..31 (pull the x+1-shift partitions down beside shift-0).
    mask = [8, 9, 10, 11, 12, 13, 14, 15, 8, 9, 10, 11, 12, 13, 14, 15,
            24, 25, 26, 27, 28, 29, 30, 31, 24, 25, 26, 27, 28, 29, 30, 31]

    with tile.TileContext(nc) as tc:
        from contextlib import ExitStack
        with ExitStack() as ctx:
            cpool = ctx.enter_context(tc.tile_pool(name="consts", bufs=1))
            tpool = ctx.enter_context(tc.tile_pool(name="tbl", bufs=1))
            ipool = ctx.enter_context(tc.tile_pool(name="idx", bufs=2))
            wpool = ctx.enter_context(tc.tile_pool(name="w", bufs=4))
            gpool = ctx.enter_context(tc.tile_pool(name="g", bufs=4))
            rpool = ctx.enter_context(tc.tile_pool(name="r", bufs=2))
            opool = ctx.enter_context(tc.tile_pool(name="o", bufs=4))
            fpool = ctx.enter_context(tc.tile_pool(name="ft", bufs=4, space="PSUM"))

            p1_t = cpool.tile([128, 80], bf, tag="p1")
            p2_t = cpool.tile([128, 80], bf, tag="p2")
            p3_t = cpool.tile([128, 80], bf, tag="p3")
            nc.sync.dma_start(p1_t[:], p1_d[:])
            nc.sync.dma_start(p2_t[:], p2_d[:])
            nc.sync.dma_start(p3_t[:], p3_d[:])

            for ph in range(2):
                tbl = tpool.tile([128, 2 * NPAIR], bf, tag="tbl")
                nc.sync.dma_start(tbl[:], emb_d[ph][:])
                idxt = ipool.tile([128, (S // 16) * NCH], dt.int16, tag="idx")
                nc.sync.dma_start(idxt[:], idx_d[ph][:])

                for ch in range(NCH):
                    s0 = ch * S
                    wxa = wpool.tile([128, S], bf, tag="wxa")
                    wya = wpool.tile([128, S], bf, tag="wya")
                    nc.sync.dma_start(wxa[:], wxa_d[ph][:, s0:s0 + S])
                    nc.sync.dma_start(wya[:], wya_d[ph][:, s0:s0 + S])

                    g = gpool.tile([128, S, 2], bf, tag="g")
                    nc.gpsimd.ap_gather(
                        g[:], tbl[:].rearrange("p (n d) -> p n d", d=2),
                        idxt[:, ch * (S // 16):(ch + 1) * (S // 16)],
                        channels=128, num_elems=NPAIR, d=2, num_idxs=S)

                    g0 = g[:, :, 0]
                    g1 = g[:, :, 1]
                    dd = rpool.tile([128, S], bf, tag="tmp")
                    r = rpool.tile([128, S], bf, tag="r")
                    nc.vector.tensor_sub(dd[:], g1, g0)
                    nc.vector.tensor_mul(dd[:], dd[:], wya[:])
                    nc.vector.tensor_add(r[:], dd[:], g0)

                    r1 = rpool.tile([128, S], bf, tag="r1")
                    nc.vector.stream_shuffle(r1[:], r[:], mask)
                    nc.vector.tensor_sub(r1[:], r1[:], r[:])
                    nc.vector.tensor_mul(r1[:], r1[:], wxa[:])
                    out8 = rpool.tile([128, S], bf, tag="out8")
                    nc.vector.tensor_add(out8[:], r1[:], r[:])

                    outT = opool.tile([128, (S // 128) * 80], dt.float32, tag="outT")
                    for bb in range(0, S // 128, 4):
                        ft = fpool.tile([128, 4, 80], dt.float32, tag="ft")
                        for j in range(4):
                            b = bb + j
                            sl = slice(b * 128, (b + 1) * 128)
                            nc.tensor.matmul(ft[:, j, :], out8[:, sl], p1_t[:],
                                             start=True, stop=False)
                            nc.tensor.matmul(ft[:, j, :], wxa[:, sl], p2_t[:],
                                             start=False, stop=False)
                            nc.tensor.matmul(ft[:, j, :], wya[:, sl], p3_t[:],
                                             start=False, stop=True)
                        if (bb // 4) % 2 == 0:
                            nc.scalar.copy(outT[:, bb * 80:(bb + 4) * 80],
                                           ft[:].rearrange("p j c -> p (j c)"))
                        else:
                            nc.vector.tensor_copy(outT[:, bb * 80:(bb + 4) * 80],
                                                  ft[:].rearrange("p j c -> p (j c)"))

                    nc.sync.dma_start(out_d[ph, ch], outT[:])
    nc.compile()
    return nc


def _prep_tables(embeddings):
    import ml_dtypes
    # [128 maps, 16516 padded grid pts, 8 feats]
    T = np.pad(embeddings.reshape(MAPS, OFF, FEAT).astype(np.float32),
               ((0, 0), (0, 132), (0, 0))).astype(ml_dtypes.bfloat16)
    v = np.arange(2 * NPAIR)
    j = v // 2
    h = v % 2
    sec = j // (NPAIR // 2)
    jj = j % (NPAIR // 2)
    colg = 2 * jj + h + sec                       # [32768]
    pp = np.arange(128)
    cm = pp // 16
    sh = (pp % 16) // 8
    ff = pp % 8
    gidx = colg[None, :] + 128 * sh[:, None]      # [128, 32768]
    tabs = []
    for k in range(NCORES):
        per_ph = []
        for ph in range(2):
            maps = np.arange(16 * k + 8 * ph, 16 * k + 8 * ph + 8)
            tab = T[maps[cm][:, None], gidx, ff[:, None]]
            per_ph.append(np.ascontiguousarray(tab))
        tabs.append(per_ph)
    return tabs


def _prep_idx_weights(inputs):
    # inputs [B, 128, 2] -> per NC, per phase: idx [128, 64*NCH] i16,
    # wxa/wya [128, B] f32 (replicated across each 16-partition group)
    x = (inputs[..., 0].astype(np.float32) * np.float32(RES - 1)).astype(np.float32)
    y = (inputs[..., 1].astype(np.float32) * np.float32(RES - 1)).astype(np.float32)
    xi = x.astype(np.int32).astype(np.int64)          # trunc, matches reference
    yi = y.astype(np.int32).astype(np.int64)
    xf = x - xi.astype(np.float32)
    yf = y - yi.astype(np.float32)
    gg = xi * RES + yi
    par = yi & 1
    idx = ((gg - par) >> 1) + par * (NPAIR // 2)  # [B, 128] int
    out = []
    for k in range(NCORES):
        per_ph = []
        for ph in range(2):
            m0 = 16 * k + 8 * ph
            idx_m = idx[:, m0:m0 + 8]             # [B, 8]
            # wrapped: partition 16m+jj slot (ch*64 + t) <- sample 16*t+jj of chunk ch
            iw = idx_m.reshape(NCH, S // 16, 16, 8)    # [ch, t, jj, m]
            iw = iw.transpose(3, 2, 0, 1).reshape(8, 16, NCH * (S // 16))
            iw = iw.reshape(128, NCH * (S // 16), order="C")  # p = m*16+jj
            wx = np.repeat(xf[:, m0:m0 + 8].T, 16, axis=0)  # [128, B]
            wy = np.repeat(yf[:, m0:m0 + 8].T, 16, axis=0)
            import ml_dtypes
            per_ph.append((np.ascontiguousarray(iw.astype(np.int16)),
                           np.ascontiguousarray(wx.astype(ml_dtypes.bfloat16)),
                           np.ascontiguousarray(wy.astype(ml_dtypes.bfloat16))))
        out.append(per_ph)
    return out


def _selectors():
    import ml_dtypes
    p1 = np.zeros((128, 80), ml_dtypes.bfloat16)
    p2 = np.zeros((128, 80), ml_dtypes.bfloat16)
    p3 = np.zeros((128, 80), ml_dtypes.bfloat16)
    for p in range(128):
        m, q = p // 16, p % 16
        if q < 8:
            p1[p, m * 10 + q] = 1.0
        if q == 0:
            p2[p, m * 10 + 8] = 1.0
            p3[p, m * 10 + 9] = 1.0
    return p1, p2, p3


def _get_executor():
    """Build (once) a cached jit executor for the SPMD program plus metadata."""
    if "exec" in _cache:
        return _cache["exec"]
    import jax
    from jax.sharding import Mesh, PartitionSpec, NamedSharding
    from jax.experimental.shard_map import shard_map
    from concourse import mybir
    from concourse.bass2jax import (_bass_exec_p, install_neuronx_cc_hook,
                                    partition_id_tensor)
    install_neuronx_cc_hook()
    if "nc" not in _cache:
        _cache["nc"] = _build_program()
    nc = _cache["nc"]
    partition_name = nc.partition_id_tensor.name if nc.partition_id_tensor else None
    in_names, out_names, out_avals, zero_outs = [], [], [], []
    for alloc in nc.m.functions[0].allocations:
        if not isinstance(alloc, mybir.MemoryLocationSet):
            continue
        name = alloc.memorylocations[0].name
        if alloc.kind == "ExternalInput":
            if name != partition_name:
                in_names.append(name)
        elif alloc.kind == "ExternalOutput":
            out_names.append(name)
            shape = tuple(alloc.tensor_shape)
            dtype = mybir.dt.np(alloc.dtype)
            out_avals.append(jax.core.ShapedArray(shape, dtype))
            zero_outs.append(np.zeros(shape, dtype))
    n_params = len(in_names)
    n_outs = len(out_avals)
    all_in_names = list(in_names) + list(out_names) + (
        [partition_name] if partition_name else [])

    def _body(*args):
        operands = list(args)
        if partition_name is not None:
            operands.append(partition_id_tensor())
        return tuple(_bass_exec_p.bind(
            *operands, out_avals=tuple(out_avals), in_names=tuple(all_in_names),
            out_names=tuple(out_names), lowering_input_output_aliases=(),
            sim_require_finite=True, sim_require_nnan=True, nc=nc))

    devices = jax.devices()[:NCORES]
    mesh = Mesh(np.asarray(devices), ("core",))
    in_specs = (PartitionSpec("core"),) * (n_params + n_outs)
    out_specs = (PartitionSpec("core"),) * n_outs
    f = jax.jit(shard_map(_body, mesh=mesh, in_specs=in_specs,
                          out_specs=out_specs, check_rep=False), keep_unused=True)
    sharding = NamedSharding(mesh, PartitionSpec("core"))
    ex = dict(f=f, in_names=in_names, out_names=out_names, zero_outs=zero_outs,
              sharding=sharding)
    _cache["exec"] = ex
    return ex


def _device_inputs(in_maps):
    import jax
    ex = _get_executor()
    per_core = [[np.asarray(m[nm]) for nm in ex["in_names"]] for m in in_maps]
    concat_in = [np.concatenate([per_core[c][i] for c in range(NCORES)], axis=0)
                 for i in range(len(ex["in_names"]))]
    concat_zeros = [np.zeros((NCORES * z.shape[0], *z.shape[1:]), z.dtype)
                    for z in ex["zero_outs"]]
    dev_in = [jax.device_put(a, ex["sharding"]) for a in concat_in]
    dev_zeros = [jax.device_put(a, ex["sharding"]) for a in concat_zeros]
    for a in dev_in + dev_zeros:
        a.block_until_ready()
    return dev_in, dev_zeros


def _prep_in_maps(inputs, embeddings):
    tabs = _prep_tables(embeddings)
    iw = _prep_idx_weights(inputs)
    p1, p2, p3 = _selectors()
    in_maps = []
    for k in range(NCORES):
        m = {"p1": p1, "p2": p2, "p3": p3}
        for ph in range(2):
            ix, wx, wy = iw[k][ph]
            m[f"emb{ph}"] = tabs[k][ph]
            m[f"idx{ph}"] = ix
            m[f"wxa{ph}"] = wx
            m[f"wya{ph}"] = wy
        in_maps.append(m)
    return in_maps


def kernel(inputs: np.ndarray, embeddings: np.ndarray) -> np.ndarray:
    inputs = np.asarray(inputs, dtype=np.float32)
    embeddings = np.asarray(embeddings, dtype=np.float32)
    in_maps = _prep_in_maps(inputs, embeddings)
    ex = _get_executor()
    dev_in, dev_zeros = _device_inputs(in_maps)
    outs = ex["f"](*dev_in, *dev_zeros)
    for o in outs:
        o.block_until_ready()
    _cache["last_dev"] = (dev_in, dev_zeros)
    res = np.asarray(outs[ex["out_names"].index("out")])
    out = np.empty((B, MAPS, FEAT + 2), np.float32)
    per_core_shape = res.shape[0] // NCORES
    for k in range(NCORES):
        st = res[k * per_core_shape:(k + 1) * per_core_shape].reshape(
            2, NCH, 128, S // 128, PH_M, FEAT + 2)
        o = st.transpose(1, 3, 2, 0, 4, 5).reshape(B, MP_NC, FEAT + 2)
        out[:, 16 * k:16 * k + 16, :] = o
    return out


def bench_exec_ns(k_small: int = 8, k_big: int = 64, reps: int = 2) -> int:
    """Steady-state per-exec device time: chained async dispatches on
    device-resident inputs; slope between two chain lengths removes the
    fixed dispatch/launch overhead."""
    import time
    ex = _get_executor()
    dev_in, dev_zeros = _cache["last_dev"]
    f = ex["f"]
    best = {}
    for K in (k_small, k_big):
        ts = []
        for _ in range(reps):
            t0 = time.time()
            outs = tuple(dev_zeros)
            for _ in range(K):
                outs = f(*dev_in, *outs)
            for o in outs:
                o.block_until_ready()
            ts.append(time.time() - t0)
        best[K] = min(ts)
    return int((best[k_big] - best[k_small]) / (k_big - k_small) * 1e9)



# revision 18
# speedup vs baseline: 1.0071x; 1.0071x over previous
"""Bass/Trainium2 kernel for nn_DenseMap (bilinear grid-sample embedding lookup).

Strategy: shard the 128 maps across 8 NeuronCores (16 maps each, in 2 phases
of 8). Table relayout (host): per map, 8 feature columns x 2 x-shifts, each
stored as even-start and odd-start y-pair streams so ONE ap_gather index with
d=2 fetches all 4 bilinear neighbors x 8 features across the 16 SBUF
partitions owned by one GPSIMD core. Device: ap_gather + DVE lerps +
stream_shuffle + PE select-transpose merge, DMA out sample-major.
"""
import sys, os
sys.path.insert(0, "/opt/trn_rl_repo")
import numpy as np

FEAT = 8
RES = 128
OFF = RES * RES          # 16384 grid pts / map
MAPS = 128
B = 32768
NCORES = 8
MP_NC = 16               # maps per NeuronCore
PH_M = 8                 # maps per phase
S = 2048                 # samples per chunk
NCH = B // S             # 32 chunks per phase
NPAIR = OFF              # num_elems for ap_gather (8192 E-pairs + 8192 O-pairs)

_cache = {}


def _build_program():
    import concourse.bass as bass
    import concourse.tile as tile
    from concourse import bacc, mybir

    nc = bacc.Bacc("TRN2", target_bir_lowering=False, debug=False,
                   num_devices=NCORES)
    dt = mybir.dt
    bf = dt.bfloat16
    emb_d = [nc.dram_tensor(f"emb{p}", [128, 2 * NPAIR], bf,
                            kind="ExternalInput").ap() for p in range(2)]
    idx_d = [nc.dram_tensor(f"idx{p}", [128, (S // 16) * NCH], dt.int16,
                            kind="ExternalInput").ap() for p in range(2)]
    wxa_d = [nc.dram_tensor(f"wxa{p}", [128, B], bf,
                            kind="ExternalInput").ap() for p in range(2)]
    wya_d = [nc.dram_tensor(f"wya{p}", [128, B], bf,
                            kind="ExternalInput").ap() for p in range(2)]
    p1_d = nc.dram_tensor("p1", [128, 80], bf, kind="ExternalInput").ap()
    p2_d = nc.dram_tensor("p2", [128, 80], bf, kind="ExternalInput").ap()
    p3_d = nc.dram_tensor("p3", [128, 80], bf, kind="ExternalInput").ap()
    out_d = nc.dram_tensor("out", [2, NCH, 128, (S // 128) * 80], dt.float32,
                           kind="ExternalOutput").ap()

    # stream_shuffle mask: within each 32-partition quadrant, rows 0..7 <- 8..15,
    # rows 16..23 <- 24..31 (pull the x+1-shift partitions down beside shift-0).
    mask = [8, 9, 10, 11, 12, 13, 14, 15, 8, 9, 10, 11, 12, 13, 14, 15,
            24, 25, 26, 27, 28, 29, 30, 31, 24, 25, 26, 27, 28, 29, 30, 31]

    with tile.TileContext(nc) as tc:
        from contextlib import ExitStack
        with ExitStack() as ctx:
            cpool = ctx.enter_context(tc.tile_pool(name="consts", bufs=1))
            tpool = ctx.enter_context(tc.tile_pool(name="tbl", bufs=1))
            ipool = ctx.enter_context(tc.tile_pool(name="idx", bufs=2))
            wpool = ctx.enter_context(tc.tile_pool(name="w", bufs=4))
            gpool = ctx.enter_context(tc.tile_pool(name="g", bufs=4))
            rpool = ctx.enter_context(tc.tile_pool(name="r", bufs=2))
            opool = ctx.enter_context(tc.tile_pool(name="o", bufs=4))
            fpool = ctx.enter_context(tc.tile_pool(name="ft", bufs=4, space="PSUM"))

            p1_t = cpool.tile([128, 80], bf, tag="p1")
            p2_t = cpool.tile([128, 80], bf, tag="p2")
            p3_t = cpool.tile([128, 80], bf, tag="p3")
            nc.sync.dma_start(p1_t[:], p1_d[:])
            nc.sync.dma_start(p2_t[:], p2_d[:])
            nc.sync.dma_start(p3_t[:], p3_d[:])

            for ph in range(2):
                tbl = tpool.tile([128, 2 * NPAIR], bf, tag="tbl")
                nc.sync.dma_start(tbl[:], emb_d[ph][:])
                idxt = ipool.tile([128, (S // 16) * NCH], dt.int16, tag="idx")
                nc.sync.dma_start(idxt[:], idx_d[ph][:])

                for ch in range(NCH):
                    s0 = ch * S
                    wxa = wpool.tile([128, S], bf, tag="wxa")
                    wya = wpool.tile([128, S], bf, tag="wya")
                    nc.sync.dma_start(wxa[:], wxa_d[ph][:, s0:s0 + S])
                    nc.sync.dma_start(wya[:], wya_d[ph][:, s0:s0 + S])

                    g = gpool.tile([128, S, 2], bf, tag="g")
                    nc.gpsimd.ap_gather(
                        g[:], tbl[:].rearrange("p (n d) -> p n d", d=2),
                        idxt[:, ch * (S // 16):(ch + 1) * (S // 16)],
                        channels=128, num_elems=NPAIR, d=2, num_idxs=S)

                    g0 = g[:, :, 0]
                    g1 = g[:, :, 1]
                    dd = rpool.tile([128, S], bf, tag="tmp")
                    r = rpool.tile([128, S], bf, tag="r")
                    nc.vector.tensor_sub(dd[:], g1, g0)
                    nc.vector.tensor_mul(dd[:], dd[:], wya[:])
                    nc.vector.tensor_add(r[:], dd[:], g0)

                    r1 = rpool.tile([128, S], bf, tag="r1")
                    nc.vector.stream_shuffle(r1[:], r[:], mask)
                    nc.vector.tensor_sub(r1[:], r1[:], r[:])
                    nc.vector.tensor_mul(r1[:], r1[:], wxa[:])
                    out8 = rpool.tile([128, S], bf, tag="out8")
                    nc.vector.tensor_add(out8[:], r1[:], r[:])

                    outT = opool.tile([128, (S // 128) * 80], dt.float32, tag="outT")
                    for bb in range(0, S // 128, 4):
                        ft = fpool.tile([128, 4, 80], dt.float32, tag="ft")
                        for j in range(4):
                            b = bb + j
                            sl = slice(b * 128, (b + 1) * 128)
                            nc.tensor.matmul(ft[:, j, :], out8[:, sl], p1_t[:],
                                             start=True, stop=False)
                            nc.tensor.matmul(ft[:, j, :], wxa[:, sl], p2_t[:],
                                             start=False, stop=False)
                            nc.tensor.matmul(ft[:, j, :], wya[:, sl], p3_t[:],
                                             start=False, stop=True)
                        nc.scalar.copy(outT[:, bb * 80:(bb + 4) * 80],
                                       ft[:].rearrange("p j c -> p (j c)"))

                    nc.sync.dma_start(out_d[ph, ch], outT[:])
    nc.compile()
    return nc


def _prep_tables(embeddings):
    import ml_dtypes
    # [128 maps, 16516 padded grid pts, 8 feats]
    T = np.pad(embeddings.reshape(MAPS, OFF, FEAT).astype(np.float32),
               ((0, 0), (0, 132), (0, 0))).astype(ml_dtypes.bfloat16)
    v = np.arange(2 * NPAIR)
    j = v // 2
    h = v % 2
    sec = j // (NPAIR // 2)
    jj = j % (NPAIR // 2)
    colg = 2 * jj + h + sec                       # [32768]
    pp = np.arange(128)
    cm = pp // 16
    sh = (pp % 16) // 8
    ff = pp % 8
    gidx = colg[None, :] + 128 * sh[:, None]      # [128, 32768]
    tabs = []
    for k in range(NCORES):
        per_ph = []
        for ph in range(2):
            maps = np.arange(16 * k + 8 * ph, 16 * k + 8 * ph + 8)
            tab = T[maps[cm][:, None], gidx, ff[:, None]]
            per_ph.append(np.ascontiguousarray(tab))
        tabs.append(per_ph)
    return tabs


def _prep_idx_weights(inputs):
    # inputs [B, 128, 2] -> per NC, per phase: idx [128, 64*NCH] i16,
    # wxa/wya [128, B] f32 (replicated across each 16-partition group)
    x = (inputs[..., 0].astype(np.float32) * np.float32(RES - 1)).astype(np.float32)
    y = (inputs[..., 1].astype(np.float32) * np.float32(RES - 1)).astype(np.float32)
    xi = x.astype(np.int32).astype(np.int64)          # trunc, matches reference
    yi = y.astype(np.int32).astype(np.int64)
    xf = x - xi.astype(np.float32)
    yf = y - yi.astype(np.float32)
    gg = xi * RES + yi
    par = yi & 1
    idx = ((gg - par) >> 1) + par * (NPAIR // 2)  # [B, 128] int
    out = []
    for k in range(NCORES):
        per_ph = []
        for ph in range(2):
            m0 = 16 * k + 8 * ph
            idx_m = idx[:, m0:m0 + 8]             # [B, 8]
            # wrapped: partition 16m+jj slot (ch*64 + t) <- sample 16*t+jj of chunk ch
            iw = idx_m.reshape(NCH, S // 16, 16, 8)    # [ch, t, jj, m]
            iw = iw.transpose(3, 2, 0, 1).reshape(8, 16, NCH * (S // 16))
            iw = iw.reshape(128, NCH * (S // 16), order="C")  # p = m*16+jj
            wx = np.repeat(xf[:, m0:m0 + 8].T, 16, axis=0)  # [128, B]
            wy = np.repeat(yf[:, m0:m0 + 8].T, 16, axis=0)
            import ml_dtypes
            per_ph.append((np.ascontiguousarray(iw.astype(np.int16)),
                           np.ascontiguousarray(wx.astype(ml_dtypes.bfloat16)),
                           np.ascontiguousarray(wy.astype(ml_dtypes.bfloat16))))
        out.append(per_ph)
    return out


def _selectors():
    import ml_dtypes
    p1 = np.zeros((128, 80), ml_dtypes.bfloat16)
    p2 = np.zeros((128, 80), ml_dtypes.bfloat16)
    p3 = np.zeros((128, 80), ml_dtypes.bfloat16)
    for p in range(128):
        m, q = p // 16, p % 16
        if q < 8:
            p1[p, m * 10 + q] = 1.0
        if q == 0:
            p2[p, m * 10 + 8] = 1.0
            p3[p, m * 10 + 9] = 1.0
    return p1, p2, p3


def _get_executor():
    """Build (once) a cached jit executor for the SPMD program plus metadata."""
    if "exec" in _cache:
        return _cache["exec"]
    import jax
    from jax.sharding import Mesh, PartitionSpec, NamedSharding
    from jax.experimental.shard_map import shard_map
    from concourse import mybir
    from concourse.bass2jax import (_bass_exec_p, install_neuronx_cc_hook,
                                    partition_id_tensor)
    install_neuronx_cc_hook()
    if "nc" not in _cache:
        _cache["nc"] = _build_program()
    nc = _cache["nc"]
    partition_name = nc.partition_id_tensor.name if nc.partition_id_tensor else None
    in_names, out_names, out_avals, zero_outs = [], [], [], []
    for alloc in nc.m.functions[0].allocations:
        if not isinstance(alloc, mybir.MemoryLocationSet):
            continue
        name = alloc.memorylocations[0].name
        if alloc.kind == "ExternalInput":
            if name != partition_name:
                in_names.append(name)
        elif alloc.kind == "ExternalOutput":
            out_names.append(name)
            shape = tuple(alloc.tensor_shape)
            dtype = mybir.dt.np(alloc.dtype)
            out_avals.append(jax.core.ShapedArray(shape, dtype))
            zero_outs.append(np.zeros(shape, dtype))
    n_params = len(in_names)
    n_outs = len(out_avals)
    all_in_names = list(in_names) + list(out_names) + (
        [partition_name] if partition_name else [])

    def _body(*args):
        operands = list(args)
        if partition_name is not None:
            operands.append(partition_id_tensor())
        return tuple(_bass_exec_p.bind(
            *operands, out_avals=tuple(out_avals), in_names=tuple(all_in_names),
            out_names=tuple(out_names), lowering_input_output_aliases=(),
            sim_require_finite=True, sim_require_nnan=True, nc=nc))

    devices = jax.devices()[:NCORES]
    mesh = Mesh(np.asarray(devices), ("core",))
    in_specs = (PartitionSpec("core"),) * (n_params + n_outs)
    out_specs = (PartitionSpec("core"),) * n_outs
    f = jax.jit(shard_map(_body, mesh=mesh, in_specs=in_specs,
                          out_specs=out_specs, check_rep=False), keep_unused=True)
    sharding = NamedSharding(mesh, PartitionSpec("core"))
    ex = dict(f=f, in_names=in_names, out_names=out_names, zero_outs=zero_outs,
              sharding=sharding)
    _cache["exec"] = ex
    return ex


def _device_inputs(in_maps):
    import jax
    ex = _get_executor()
    per_core = [[np.asarray(m[nm]) for nm in ex["in_names"]] for m in in_maps]
    concat_in = [np.concatenate([per_core[c][i] for c in range(NCORES)], axis=0)
                 for i in range(len(ex["in_names"]))]
    concat_zeros = [np.zeros((NCORES * z.shape[0], *z.shape[1:]), z.dtype)
                    for z in ex["zero_outs"]]
    dev_in = [jax.device_put(a, ex["sharding"]) for a in concat_in]
    dev_zeros = [jax.device_put(a, ex["sharding"]) for a in concat_zeros]
    for a in dev_in + dev_zeros:
        a.block_until_ready()
    return dev_in, dev_zeros


def _prep_in_maps(inputs, embeddings):
    tabs = _prep_tables(embeddings)
    iw = _prep_idx_weights(inputs)
    p1, p2, p3 = _selectors()
    in_maps = []
    for k in range(NCORES):
        m = {"p1": p1, "p2": p2, "p3": p3}
        for ph in range(2):
            ix, wx, wy = iw[k][ph]
            m[f"emb{ph}"] = tabs[k][ph]
            m[f"idx{ph}"] = ix
            m[f"wxa{ph}"] = wx
            m[f"wya{ph}"] = wy
        in_maps.append(m)
    return in_maps


def kernel(inputs: np.ndarray, embeddings: np.ndarray) -> np.ndarray:
    inputs = np.asarray(inputs, dtype=np.float32)
    embeddings = np.asarray(embeddings, dtype=np.float32)
    in_maps = _prep_in_maps(inputs, embeddings)
    ex = _get_executor()
    dev_in, dev_zeros = _device_inputs(in_maps)
    outs = ex["f"](*dev_in, *dev_zeros)
    for o in outs:
        o.block_until_ready()
    _cache["last_dev"] = (dev_in, dev_zeros)
    res = np.asarray(outs[ex["out_names"].index("out")])
    out = np.empty((B, MAPS, FEAT + 2), np.float32)
    per_core_shape = res.shape[0] // NCORES
    for k in range(NCORES):
        st = res[k * per_core_shape:(k + 1) * per_core_shape].reshape(
            2, NCH, 128, S // 128, PH_M, FEAT + 2)
        o = st.transpose(1, 3, 2, 0, 4, 5).reshape(B, MP_NC, FEAT + 2)
        out[:, 16 * k:16 * k + 16, :] = o
    return out


def bench_exec_ns(k_small: int = 8, k_big: int = 64, reps: int = 2) -> int:
    """Steady-state per-exec device time: chained async dispatches on
    device-resident inputs; slope between two chain lengths removes the
    fixed dispatch/launch overhead."""
    import time
    ex = _get_executor()
    dev_in, dev_zeros = _cache["last_dev"]
    f = ex["f"]
    best = {}
    for K in (k_small, k_big):
        ts = []
        for _ in range(reps):
            t0 = time.time()
            outs = tuple(dev_zeros)
            for _ in range(K):
                outs = f(*dev_in, *outs)
            for o in outs:
                o.block_until_ready()
            ts.append(time.time() - t0)
        best[K] = min(ts)
    return int((best[k_big] - best[k_small]) / (k_big - k_small) * 1e9)

